# revision 5
# baseline (speedup 1.0000x reference)
"""Trainium2 Bass kernel for gated latent-query attention pooling.

Computation (per reference):
  x = patch @ W_proj + b_proj                  [N, 64]
  gate = sigmoid(x @ W_gate + b_gate)          [N, 1]
  keys = x @ W_k + b_k
  scores = (Lq @ keys.T) / 8 * gate.T          [4, N]  (= A_out)
  attn = softmax(scores, axis=N)
  latent = attn @ x                            [4, 64]
  logits = relu(latent.flat @ W_fc + b_fc) @ W_out + b_out

Sharding: N split across 8 cores (12500 each, zero-padded to 12800).
Each core computes its scores slice + local softmax stats (max, exp-sum)
and exp-weighted x sums; cross-core combine via AllReduce(max) +
AllReduce(add); FC head replicated on every core.

Key folds done host-side:
  A = (Lq/8) @ W_k.T  -> scores_raw = A @ x.T + (Lq/8)@b_k : keys never built.
  gate weights replicated to 4 rows so one [64,8] matmul yields scores+gates.
  ones column appended to pooled x so sum(exp) falls out of the pooling matmul.
"""

import os
import numpy as np
from contextlib import ExitStack

import concourse.bacc as bacc
import concourse.bass as bass
import concourse.tile as tile
from concourse import mybir
from concourse.bass_utils import run_bass_kernel_spmd

N_FULL, D_IN, D, Q, OUT_DIM = 100000, 1024, 64, 4, 4
NCORES = 8
N_LOC = N_FULL // NCORES          # 12500
BLK = 512
N_PAD = 12800                     # 25 blocks of 512
NBLK = N_PAD // BLK               # 25
KC = D_IN // 128                  # 8 k-chunks
NCH = N_PAD // 128                # 100 chunks for pooling
F32 = mybir.dt.float32
F32R = mybir.dt.float32r
NEG_BIG = -1e30

# matmul dtype for the big patch-side matmuls: float32r streams 4x faster on
# the PE than float32 at free-dim >= 256; flip to F32 if precision demands.
MM_DT = F32R if os.environ.get("KERNEL_MM_DT", "f32r") == "f32r" else F32

AF = mybir.ActivationFunctionType

LAST_EXEC_NS = None
_CACHE = {}


def _build_nc():
    nc = bacc.Bacc("TRN2", target_bir_lowering=False, debug=False,
                   num_devices=NCORES)

    patch = nc.dram_tensor("patch", [N_PAD, D_IN], F32, kind="ExternalInput")
    wp = nc.dram_tensor("wp", [128, KC, D], F32, kind="ExternalInput")
    bp = nc.dram_tensor("bp", [D, 1], F32, kind="ExternalInput")
    wsg = nc.dram_tensor("wsg", [D, 36], F32, kind="ExternalInput")
    csg = nc.dram_tensor("csg", [Q, 1], F32, kind="ExternalInput")
    bg = nc.dram_tensor("bg", [Q, 1], F32, kind="ExternalInput")
    eye = nc.dram_tensor("eye", [128, 128], F32, kind="ExternalInput")
    wfc = nc.dram_tensor("wfc", [D, Q, D], F32, kind="ExternalInput")
    bfc = nc.dram_tensor("bfc", [1, D], F32, kind="ExternalInput")
    wout = nc.dram_tensor("wout", [D, OUT_DIM], F32, kind="ExternalInput")
    bout = nc.dram_tensor("bout", [1, OUT_DIM], F32, kind="ExternalInput")

    aout = nc.dram_tensor("aout", [Q, N_LOC], F32, kind="ExternalOutput")
    logits = nc.dram_tensor("logits", [1, OUT_DIM], F32, kind="ExternalOutput")

    groups = [list(range(NCORES))]

    with tile.TileContext(nc) as tc, ExitStack() as top:
        const = top.enter_context(tc.tile_pool(name="const", bufs=1))
        big = top.enter_context(tc.tile_pool(name="big", bufs=1))
        pers = top.enter_context(tc.tile_pool(name="pers", bufs=1))
        dram = top.enter_context(tc.tile_pool(name="dram", bufs=1, space="DRAM"))

        wp_s = const.tile([128, KC, D], F32)
        nc.sync.dma_start(wp_s[:], wp[:])
        bp_s = const.tile([D, 1], F32)
        nc.sync.dma_start(bp_s[:], bp[:])
        wsg_s = const.tile([D, 36], F32)
        nc.sync.dma_start(wsg_s[:], wsg[:])
        csg_s = const.tile([Q, 1], F32)
        nc.sync.dma_start(csg_s[:], csg[:])
        bg_s = const.tile([Q, 1], F32)
        nc.sync.dma_start(bg_s[:], bg[:])
        eye_s = const.tile([128, 128], F32)
        nc.sync.dma_start(eye_s[:], eye[:])
        if MM_DT == F32R:
            wp_r = const.tile([128, KC, D], F32R)
            nc.vector.tensor_copy(wp_r[:], wp_s[:])
            wsg_r = const.tile([D, 36], F32R)
            nc.vector.tensor_copy(wsg_r[:], wsg_s[:])
            eye_r = const.tile([128, 128], F32R)
            nc.vector.tensor_copy(eye_r[:], eye_s[:])
        else:
            wp_r, wsg_r, eye_r = wp_s, wsg_s, eye_s
        wfc_s = const.tile([D, Q, D], F32)
        nc.sync.dma_start(wfc_s[:], wfc[:])
        bfc_s = const.tile([1, D], F32)
        nc.sync.dma_start(bfc_s[:], bfc[:])
        wout_s = const.tile([D, OUT_DIM], F32)
        nc.sync.dma_start(wout_s[:], wout[:])
        bout_s = const.tile([1, OUT_DIM], F32)
        nc.sync.dma_start(bout_s[:], bout[:])

        # x with n on partitions, +ones column for the exp-sum; scores [4, n]
        xn = big.tile([128, NCH, D + 2], MM_DT)
        sco = big.tile([Q, N_PAD], F32)
        stats = big.tile([Q, NBLK], F32)
        # fp32r memset is not a legal ISA op: stage fp32 then round via ACT.
        onz = big.tile([128, NCH, 2], F32)
        nc.vector.memset(onz[:, :, 0:1], 1.0)
        nc.vector.memset(onz[:, :, 1:2], 0.0)
        nc.scalar.copy(xn[:, :, D:D + 2], onz[:])
        nc.vector.memset(sco[:, N_LOC:], NEG_BIG)

        # ---------------- pass A: project, score, gate ----------------
        with ExitStack() as pa:
            iop = pa.enter_context(tc.tile_pool(name="io", bufs=2))
            ptp = pa.enter_context(tc.tile_pool(name="ptT", bufs=2))
            xtp = pa.enter_context(tc.tile_pool(name="xt", bufs=3))
            smp = pa.enter_context(tc.tile_pool(name="smA", bufs=2))
            ps_tr = pa.enter_context(
                tc.tile_pool(name="ps_tr", bufs=3, space="PSUM"))
            ps_xt = pa.enter_context(
                tc.tile_pool(name="ps_xt", bufs=2, space="PSUM"))
            ps_sg = pa.enter_context(
                tc.tile_pool(name="ps_sg", bufs=1, space="PSUM"))
            ps_xn = pa.enter_context(
                tc.tile_pool(name="ps_xn", bufs=2, space="PSUM"))

            for b in range(NBLK):
                n0 = b * BLK
                nv = min(BLK, N_LOC - n0)  # valid (unpadded) cols this block

                pt = iop.tile([128, 4, D_IN], F32)
                src = patch[n0:n0 + BLK, :].rearrange("(a p) k -> p a k", p=128)
                nc.sync.dma_start(pt[:], src)

                ptT = ptp.tile([128, KC, BLK], MM_DT)
                ps_x = ps_xt.tile([D, BLK], F32)
                for kc in range(KC):
                    tr = ps_tr.tile([128, BLK], F32)
                    for j in range(4):
                        nc.tensor.transpose(
                            tr[:, j * 128:(j + 1) * 128],
                            pt[:, j, kc * 128:(kc + 1) * 128],
                            eye_s[:],
                        )
                    if kc % 2 == 0:
                        nc.scalar.copy(ptT[:, kc, :], tr[:])
                    else:
                        nc.vector.tensor_copy(ptT[:, kc, :], tr[:])
                    nc.tensor.matmul(
                        ps_x[:], wp_r[:, kc, :], ptT[:, kc, :],
                        start=(kc == 0), stop=(kc == KC - 1),
                    )

                xt = xtp.tile([D, BLK], MM_DT)
                nc.scalar.add(xt[:], ps_x[:], bp_s[:])  # x.T = proj + b_proj

                ps_s = ps_sg.tile([36, BLK], F32)
                nc.tensor.matmul(ps_s[:], wsg_r[:], xt[:],
                                 start=True, stop=True)

                g4 = smp.tile([Q, BLK], F32)
                nc.scalar.activation(g4[:], ps_s[32:36, :], AF.Sigmoid,
                                     bias=bg_s[:])
                tmp = smp.tile([Q, BLK], F32)
                nc.vector.tensor_scalar_add(tmp[:, :nv], ps_s[0:Q, :nv],
                                            csg_s[:])
                nc.vector.tensor_mul(sco[:, n0:n0 + nv], tmp[:, :nv],
                                     g4[:, :nv])
                nc.vector.reduce_max(stats[:, b:b + 1], sco[:, n0:n0 + nv],
                                     axis=mybir.AxisListType.X)

                for j in range(4):
                    psn = ps_xn.tile([128, D], MM_DT)
                    nc.tensor.transpose(psn[:], xt[:, j * 128:(j + 1) * 128],
                                        eye_r[:D, :D])
                    ci = b * 4 + j
                    if j % 2 == 0:
                        nc.scalar.copy(xn[:, ci, 0:D], psn[:])
                    else:
                        nc.vector.tensor_copy(xn[:, ci, 0:D], psn[:])

        # A_out slice out
        nc.sync.dma_start(aout[:], sco[:, :N_LOC])

        # local max, kick off the max-allreduce early so it overlaps pass B
        mloc = pers.tile([Q, 1], F32)
        nc.vector.reduce_max(mloc[:], stats[:], axis=mybir.AxisListType.X)
        m_in = dram.tile([Q, 1], F32)
        m_out = dram.tile([Q, 1], F32, addr_space="Shared")
        nc.sync.dma_start(m_in[:], mloc[:])
        nc.gpsimd.collective_compute(
            "AllReduce", mybir.AluOpType.max, replica_groups=groups,
            ins=[m_in.opt()], outs=[m_out.opt()])

        negm = pers.tile([Q, 1], F32)
        nc.vector.tensor_scalar_mul(negm[:], mloc[:], -1.0)

        # ---------------- pass B: exp + pooling ----------------
        with ExitStack() as pb:
            ptsb = pb.enter_context(tc.tile_pool(name="pts", bufs=3))
            ps_p4 = pb.enter_context(
                tc.tile_pool(name="ps_p4", bufs=2, space="PSUM"))
            ps_pool = pb.enter_context(
                tc.tile_pool(name="ps_pool", bufs=1, space="PSUM"))

            ps_P = ps_pool.tile([Q, D + 2], F32)
            for sb in range(NBLK):
                c0 = sb * BLK
                nc.scalar.activation(sco[:, c0:c0 + BLK], sco[:, c0:c0 + BLK],
                                     AF.Exp, bias=negm[:])
                p4 = ps_p4.tile([128, 4, Q], F32)
                for j in range(4):
                    nc.tensor.transpose(
                        p4[:, j, :],
                        sco[:, c0 + j * 128:c0 + (j + 1) * 128],
                        eye_s[:Q, :Q],
                    )
                pts = ptsb.tile([128, 4, Q], MM_DT)
                nc.vector.tensor_copy(pts[:], p4[:])
                for j in range(4):
                    c = sb * 4 + j
                    nc.tensor.matmul(ps_P[:], pts[:, j, :], xn[:, c, :],
                                     start=(c == 0), stop=(c == NCH - 1))

            # cross-core combine
            mg = pers.tile([Q, 1], F32)
            nc.sync.dma_start(mg[:], m_out[:])
            dm = pers.tile([Q, 1], F32)
            nc.vector.tensor_sub(dm[:], mloc[:], mg[:])
            al = pers.tile([Q, 1], F32)
            nc.scalar.activation(al[:], dm[:], AF.Exp)
            sp = pers.tile([Q, D + 2], F32)
            nc.vector.tensor_scalar_mul(sp[:], ps_P[:], al[:])

        sp_in = dram.tile([Q, D + 2], F32)
        sp_out = dram.tile([Q, D + 2], F32, addr_space="Shared")
        nc.sync.dma_start(sp_in[:], sp[:])
        nc.gpsimd.collective_compute(
            "AllReduce", mybir.AluOpType.add, replica_groups=groups,
            ins=[sp_in.opt()], outs=[sp_out.opt()])

        # ---------------- replicated FC head ----------------
        with ExitStack() as pt_:
            ps_t = pt_.enter_context(
                tc.tile_pool(name="ps_tail", bufs=1, space="PSUM"))
            ps_t2 = pt_.enter_context(
                tc.tile_pool(name="ps_tail2", bufs=1, space="PSUM"))

            spg = pers.tile([Q, D + 2], F32)
            nc.sync.dma_start(spg[:], sp_out[:])
            rS = pers.tile([Q, 1], F32)
            nc.vector.reciprocal(rS[:], spg[:, D:D + 1])
            lat = pers.tile([Q, D], F32)
            nc.vector.tensor_scalar_mul(lat[:], spg[:, 0:D], rS[:])

            ps_lt = ps_t.tile([D, Q], F32)
            nc.tensor.transpose(ps_lt[:], lat[:], eye_s[:Q, :Q])
            latT = pers.tile([D, Q], F32)
            nc.vector.tensor_copy(latT[:], ps_lt[:])

            ps_h = ps_t2.tile([1, D], F32)
            for q in range(Q):
                nc.tensor.matmul(ps_h[:], latT[:, q:q + 1], wfc_s[:, q, :],
                                 start=(q == 0), stop=(q == Q - 1))
            h = pers.tile([1, D], F32)
            nc.vector.tensor_add(h[:], ps_h[:], bfc_s[:])
            hr = pers.tile([1, D], F32)
            nc.scalar.activation(hr[:], h[:], AF.Relu)

            ps_hT = ps_t.tile([D, 1], F32)
            nc.tensor.transpose(ps_hT[:], hr[:], eye_s[:1, :1])
            hT = pers.tile([D, 1], F32)
            nc.vector.tensor_copy(hT[:], ps_hT[:])

            ps_lg = ps_t2.tile([1, OUT_DIM], F32)
            nc.tensor.matmul(ps_lg[:], hT[:], wout_s[:], start=True, stop=True)
            lg = pers.tile([1, OUT_DIM], F32)
            nc.vector.tensor_add(lg[:], ps_lg[:], bout_s[:])
            nc.sync.dma_start(logits[:], lg[:])

    nc.compile()
    return nc


def _host_inputs(patch_features, latent_queries, W_proj, b_proj, W_k, b_k,
                 W_gate, b_gate, W_fc, b_fc, W_out, b_out):
    f = np.float32
    lq_s = (np.asarray(latent_queries, f) / np.sqrt(np.float32(D))).astype(f)
    W_k = np.asarray(W_k, f)
    wsg = np.zeros((D, 36), f)
    wsg[:, :Q] = W_k @ lq_s.T                       # A.T
    wsg[:, 32:36] = np.repeat(np.asarray(W_gate, f), Q, axis=1)
    csg = (lq_s @ np.asarray(b_k, f)).reshape(Q, 1).astype(f)
    bg = np.full((Q, 1), np.asarray(b_gate, f).reshape(-1)[0], f)
    wp = np.asarray(W_proj, f).reshape(KC, 128, D).transpose(1, 0, 2).copy()
    wfc = np.asarray(W_fc, f).reshape(Q, D, D).transpose(1, 0, 2).copy()

    shared = {
        "wp": wp,
        "bp": np.asarray(b_proj, f).reshape(D, 1).copy(),
        "wsg": wsg,
        "csg": csg,
        "bg": bg,
        "eye": np.eye(128, dtype=f),
        "wfc": wfc,
        "bfc": np.asarray(b_fc, f).reshape(1, D).copy(),
        "wout": np.asarray(W_out, f).copy(),
        "bout": np.asarray(b_out, f).reshape(1, OUT_DIM).copy(),
    }

    patch = np.asarray(patch_features, f)
    in_maps = []
    for c in range(NCORES):
        sl = patch[c * N_LOC:(c + 1) * N_LOC]
        pad = np.zeros((N_PAD, D_IN), f)
        pad[:N_LOC] = sl
        in_maps.append({"patch": pad, **shared})
    return in_maps


def kernel(patch_features, mask, latent_queries, W_proj, b_proj, W_k, b_k,
           W_gate, b_gate, W_fc, b_fc, W_out, b_out):
    global LAST_EXEC_NS
    if "nc" not in _CACHE:
        _CACHE["nc"] = _build_nc()
    nc = _CACHE["nc"]

    in_maps = _host_inputs(patch_features, latent_queries, W_proj, b_proj,
                           W_k, b_k, W_gate, b_gate, W_fc, b_fc, W_out, b_out)
    res = run_bass_kernel_spmd(nc, in_maps, list(range(NCORES)))
    LAST_EXEC_NS = res.exec_time_ns

    logits = np.asarray(res.results[0]["logits"], np.float32).reshape(1, OUT_DIM)
    a_out = np.concatenate(
        [np.asarray(res.results[c]["aout"], np.float32) for c in range(NCORES)],
        axis=1,
    )[None]  # [1, Q, N]
    return logits, a_out
